# revision 49
# baseline (speedup 1.0000x reference)
"""MultiHeadRichAttention Trainium2 Bass kernel (8-core data parallel).

Math (per batch b, folding done host-side):
  x = [keys, q, keys*q, keys-q] @ W1f  ==  [keysT; (keys*q)T] @ W12 + C[b]
      where W12 = [W1A+W1D; W1C], C = q @ (W1B - W1D) + b1
  H1 = prelu(mm1 + C, a1)      C added as per-partition bias in the PReLU
  H2 = prelu(H1 @ W2bd + b2, a2)
  scores = H2 @ W3bd  (+ mask, fused into the score PSUM accumulation via
      a K=4 selector matmul; b3 dropped: softmax-invariant)
  w = softmax_masked(scores); wbar = mean_h w
  out = wbar @ (keys @ Wo + bo)   (bo fold exact: sum_s wbar = 1)

Per-core layout: 256 batches = 128 pairs = 32 bank-groups of 8 batches.
Scores col-packed into one PSUM bank per group via tile_position; softmax
batched on [128, 400] tiles with exp row-sums fused via accum_out.
Transpose+head-mean+normalize fused into 4 tiny matmuls: stationary =
exp-scores chunk, moving = E [128,4] block-diagonal of 1/(4Z) (built per
half by GpSimd from the reciprocal column).  Final per batch with [s,1]
stationaries into a single [1,512] PSUM row.
DRAM layouts host-packed so every DMA lands >=800B contiguous/partition.
"""
import numpy as np
import ml_dtypes

import concourse.bass as bass
import concourse.bacc as bacc
import concourse.tile as tile
from concourse import mybir
from concourse.bass_utils import run_bass_kernel_spmd

F32 = mybir.dt.float32
BF16 = mybir.dt.bfloat16
AX = mybir.AxisListType
ALU = mybir.AluOpType
ACTF = mybir.ActivationFunctionType

NCORES = 8
B, S, D, H = 2048, 200, 64, 4
H1N, H2N = 64, 32
BL = B // NCORES          # 256 batches per core
NPAIR = BL // 2           # 128
NGRP = NPAIR // 4         # 32 bank-groups (8 batches each)
SC0, SC1 = 128, S - 128   # s-chunks 128 + 72

bf16 = ml_dtypes.bfloat16


def _register_prelu_bias_op():
    import concourse.dve_ops as dve_ops
    from concourse.dve_ops import DveOp, OPS, CUSTOM_DVE_SPECS, _SUB_OPCODE_FOR_NAME
    from concourse.dve_spec import Spec, Src0, C0, C1, maxx, lower
    from concourse.dve_uop import DveOpSpec

    if "PRELU_B_ANT" in CUSTOM_DVE_SPECS:
        return next(op for op in OPS if op.name == "PRELU_B_ANT")
    spec = Spec(
        body=maxx(Src0 + C1, (Src0 + C1) * C0),
        reference=lambda in0, in1, s0, s1, imm2: np.maximum(
            in0.astype(np.float32) + s1, (in0.astype(np.float32) + s1) * s0
        ),
    )
    row = dve_ops._CUSTOM_DVE_ROW_BASE + len(OPS)
    shas = {}
    for ver in ("v3", "v4"):
        try:
            tmp = DveOpSpec(name="PRELU_B_ANT", opcode=row,
                            uops=lower(spec, ver=ver), rd1_en=False)
            shas[ver] = tmp.sha(ver)
        except Exception:
            pass
    op = DveOp("PRELU_B_ANT", spec, subdim=False, uops_sha=shas)
    OPS.append(op)
    CUSTOM_DVE_SPECS["PRELU_B_ANT"] = spec
    _SUB_OPCODE_FOR_NAME["PRELU_B_ANT"] = row
    return op


def build_nc():
    """Build the per-core Bass program (same program on all 8 cores)."""
    from contextlib import ExitStack

    PRELU_B = _register_prelu_bias_op()
    nc = bacc.Bacc("TRN2", target_bir_lowering=False, debug=False,
                   num_devices=NCORES)

    # host-packed DRAM layouts (see prep_inputs)
    x2g_d = nc.dram_tensor("x2g", [NGRP, 128, 1600], BF16,
                           kind="ExternalInput").ap()
    kng_d = nc.dram_tensor("kng", [NGRP, S, 512], BF16,
                           kind="ExternalInput").ap()
    mgp_d = nc.dram_tensor("mgp", [NGRP, 4, 400], BF16,
                           kind="ExternalInput").ap()
    # packed bf16 constants: W12 0:256 | w2_0 256:384 | w2_1 384:512 |
    # w3pad 512:544 | sel01 544:672 | m01 672:676
    cb_d = nc.dram_tensor("cb", [128, 676], BF16, kind="ExternalInput").ap()
    # f32 per-batch bias table: col 256*c + b = C[b, 128c + p]
    ctb_d = nc.dram_tensor("ctb", [128, 512], F32, kind="ExternalInput").ap()
    # pv (f32): 0 a1c0 | 1 zeros | 2 a1c1 | 3 a2 | 4 b2 | 5 spare
    pv_d = nc.dram_tensor("pv", [128, 6], F32, kind="ExternalInput").ap()
    out_d = nc.dram_tensor("out", [BL, D], F32, kind="ExternalOutput").ap()

    with tile.TileContext(nc) as tc, ExitStack() as ctx:
        const = ctx.enter_context(tc.tile_pool(name="const", bufs=1))
        x2p = ctx.enter_context(tc.tile_pool(name="x2p", bufs=3))
        h1p = ctx.enter_context(tc.tile_pool(name="h1p", bufs=4))
        h2p = ctx.enter_context(tc.tile_pool(name="h2p", bufs=4))
        smp = ctx.enter_context(tc.tile_pool(name="smp", bufs=3))
        knwp = ctx.enter_context(tc.tile_pool(name="knwp", bufs=3))
        mpp = ctx.enter_context(tc.tile_pool(name="mpp", bufs=3))
        wbtp = ctx.enter_context(tc.tile_pool(name="wbtp", bufs=3))
        pfcp = ctx.enter_context(tc.tile_pool(name="pfcp", bufs=2))
        p1p = ctx.enter_context(tc.tile_pool(name="p1p", bufs=4, space="PSUM"))
        p2p = ctx.enter_context(tc.tile_pool(name="p2p", bufs=1, space="PSUM"))
        scbp = ctx.enter_context(tc.tile_pool(name="scbp", bufs=1, space="PSUM"))
        wtp = ctx.enter_context(tc.tile_pool(name="wtp", bufs=1, space="PSUM"))
        pfp = ctx.enter_context(tc.tile_pool(name="pfp", bufs=1, space="PSUM"))

        cb_t = const.tile([128, 676], BF16)
        ctb_t = const.tile([128, 512], F32)
        pv_t = const.tile([128, 6], F32)
        nc.sync.dma_start(out=cb_t, in_=cb_d)
        nc.sync.dma_start(out=ctb_t, in_=ctb_d)
        nc.sync.dma_start(out=pv_t, in_=pv_d)
        w12_t = cb_t[:, 0:256]
        w2_t = [cb_t[:, 256:384], cb_t[:, 384:512]]
        w3_t = cb_t[:, 512:544]
        sel01_t = cb_t[0:4, 544:672]
        m01_t = cb_t[:, 672:676]
        a1_t = [pv_t[:, 0:1], pv_t[:, 2:3]]
        zb_t = pv_t[:, 1:2]
        a2_t = pv_t[:, 3:4]
        b2_t = pv_t[:, 4:5]

        def emit_tail_a(g, e_t, eb_t, kn0, kn1):
            """E-matmuls of group g — emitted one group late so the
            in-order PE queue never waits on g's softmax chain."""
            # fused transpose+head-mean+normalize:
            #   wbtP[s, 8*sc+4*bb+j] = sum_i e[32j+i, s] / (4 Z)
            wbtP = wtp.tile([128, 16], F32, tag="wbtP", name="wbtP")
            for bb in range(2):
                for sc, (c0, clen) in enumerate([(0, SC0), (SC0, SC1)]):
                    nc.tensor.matmul(
                        wbtP[0:clen, 8 * sc + 4 * bb:8 * sc + 4 * bb + 4],
                        e_t[:, S * bb + c0:S * bb + c0 + clen],
                        eb_t[:, 4 * bb:4 * bb + 4],
                        start=True, stop=True, skip_group_check=True)
            wbt_s = wbtp.tile([128, 16], BF16, tag="wbt_s", name="wbt_s")
            nc.scalar.copy(wbt_s, wbtP)
            return wbt_s

        def emit_tail_b(g, wbt_s, kn0, kn1):
            """Final matmuls + output of group g — emitted later still, so
            the wbt_s copy has cleared the scalar queue."""
            b0 = 8 * g
            pf_t = pfp.tile([1, 512], F32, tag="pf", name="pf")
            for bg in range(8):
                j, bb = bg // 2, bg % 2
                nc.tensor.matmul(pf_t[0:1, D * bg:D * (bg + 1)],
                                 wbt_s[0:SC0, 4 * bb + j:4 * bb + j + 1],
                                 kn0[:, D * bg:D * (bg + 1)],
                                 start=True, stop=False)
                nc.tensor.matmul(pf_t[0:1, D * bg:D * (bg + 1)],
                                 wbt_s[0:SC1, 8 + 4 * bb + j:8 + 4 * bb + j + 1],
                                 kn1[:, D * bg:D * (bg + 1)],
                                 start=False, stop=True)
            pfc_t = pfcp.tile([1, 512], F32, tag="pfc", name="pfc")
            nc.scalar.copy(pfc_t, pf_t)
            nc.sync.dma_start(
                out=out_d[b0:b0 + 8, :].rearrange("b d -> (b d)")[None, :],
                in_=pfc_t)

        tails = []                      # 2-group-delayed PE tails
        for g in range(NGRP):
            b0 = 8 * g
            # ---- group DMAs (all contiguous >=800B per partition) ----
            x2_t = x2p.tile([128, 1600], BF16, tag="x2", name="x2")
            nc.sync.dma_start(out=x2_t, in_=x2g_d[g])
            kn0 = knwp.tile([SC0, 512], BF16, tag="kn0", name="kn0")
            kn1 = knwp.tile([SC1, 512], BF16, tag="kn1", name="kn1")
            nc.sync.dma_start(out=kn0, in_=kng_d[g, 0:SC0])
            nc.sync.dma_start(out=kn1, in_=kng_d[g, SC0:S])
            mg_t = mpp.tile([4, 400], BF16, tag="mg", name="mg")
            nc.sync.dma_start(out=mg_t, in_=mgp_d[g])

            scb_t = scbp.tile([128, 2 * S], F32, tag="scb", name="scb")
            h2_prev = None              # delayed W3-matmul operand
            for lp4 in range(4):
                bp = b0 + 2 * lp4       # first batch of this pair
                x2s = x2_t[:, 400 * lp4:400 * (lp4 + 1)]
                h1_ts = []
                for c in range(2):
                    p1_t = p1p.tile([128, 2 * S], F32, tag="p1", name="p1")
                    nc.tensor.matmul(p1_t, w12_t[:, 128 * c:128 * (c + 1)],
                                     x2s, start=True, stop=True)
                    if lp4 == 0 and c == 1 and len(tails) >= 2:
                        tg, e_p, eb_p, kn0_p, kn1_p = tails.pop(0)
                        wbt_s = emit_tail_a(tg, e_p, eb_p, kn0_p, kn1_p)
                        emit_tail_b(tg, wbt_s, kn0_p, kn1_p)
                    h1_t = h1p.tile([128, 2 * S], BF16, tag=f"h1_{c}",
                                    name=f"h1_{c}")
                    for bb in range(2):
                        sl = slice(S * bb, S * (bb + 1))
                        bias = ctb_t[:, 256 * c + bp + bb:256 * c + bp + bb + 1]
                        # engine balance: 8 halves scalar, 8 vector
                        if c == 0:
                            nc.scalar.activation(h1_t[:, sl], p1_t[:, sl],
                                                 ACTF.Prelu, bias=bias,
                                                 alpha=a1_t[c])
                        else:
                            nc.vector._custom_dve(PRELU_B, out=h1_t[:, sl],
                                                  in0=p1_t[:, sl],
                                                  s0=a1_t[c], s1=bias)
                    h1_ts.append(h1_t)

                p2_t = p2p.tile([128, 2 * S], F32, tag="p2", name="p2")
                nc.tensor.matmul(p2_t, w2_t[0], h1_ts[0], start=True, stop=False)
                nc.tensor.matmul(p2_t, w2_t[1], h1_ts[1], start=False, stop=True)
                if lp4 == 1:
                    # mask seeds the score accumulation; emitted mid-group so
                    # the PE never waits on the previous group's exp reads
                    nc.tensor.matmul(scb_t, sel01_t, mg_t, start=True,
                                     stop=False, skip_group_check=True)
                if h2_prev is not None:
                    lpp = lp4 - 1
                    nc.tensor.matmul(scb_t[32 * lpp:32 * (lpp + 1), :], w3_t,
                                     h2_prev, start=False, stop=False,
                                     tile_position=(0, 32 * lpp),
                                     skip_group_check=True)
                h2_t = h2p.tile([128, 2 * S], BF16, tag="h2", name="h2")
                if lp4 == 0:
                    nc.scalar.activation(h2_t, p2_t, ACTF.Prelu,
                                         bias=b2_t, alpha=a2_t)
                else:
                    nc.vector._custom_dve(PRELU_B, out=h2_t, in0=p2_t,
                                          s0=a2_t, s1=b2_t)
                h2_prev = h2_t
            nc.tensor.matmul(scb_t[96:128, :], w3_t, h2_prev,
                             start=False, stop=True, tile_position=(0, 96),
                             skip_group_check=True)

            # ---- softmax over the group's score bank ----
            e_t = smp.tile([128, 2 * S], BF16, tag="e", name="e")
            nc.scalar.activation(e_t, scb_t, ACTF.Exp, bias=zb_t)
            ss_t = wbtp.tile([128, 2], F32, tag="ss", name="ss")
            nc.vector.tensor_reduce(
                out=ss_t, in_=e_t.rearrange("p (t s) -> p t s", t=2),
                axis=AX.X, op=ALU.add, opt_input=False)
            ss5_t = wbtp.tile([128, 2], F32, tag="ss5", name="ss5")
            nc.gpsimd.tensor_scalar(ss5_t, ss_t, 1e-30, 4.0, ALU.max, ALU.mult)
            r4_t = wbtp.tile([128, 2], F32, tag="r4", name="r4")
            nc.vector.reciprocal(r4_t, ss5_t)
            # E [128, (bb 4)] bf16: block-diag 1/(4Z) per head-row (GpSimd)
            eb_t = wbtp.tile([128, 8], BF16, tag="eb", name="eb")
            for bb in range(2):
                nc.gpsimd.tensor_scalar_mul(eb_t[:, 4 * bb:4 * bb + 4],
                                            m01_t, r4_t[:, bb:bb + 1])
            tails.append((g, e_t, eb_t, kn0, kn1))
        wbts = [emit_tail_a(tg, e_p, eb_p, kn0_p, kn1_p)
                for tg, e_p, eb_p, kn0_p, kn1_p in tails]
        for (tg, e_p, eb_p, kn0_p, kn1_p), wbt_s in zip(tails, wbts):
            emit_tail_b(tg, wbt_s, kn0_p, kn1_p)
    nc.compile()
    return nc


def prep_inputs(query, keys, keys_mask, W1, b1, a1, W2, b2, a2, W3, b3, Wo, bo):
    """Host-side folding; returns per-core in_maps."""
    q = np.asarray(query, np.float32)
    keys = np.asarray(keys, np.float32)
    mask = np.asarray(keys_mask)
    W1 = np.asarray(W1, np.float32)
    W1f = np.transpose(W1, (1, 0, 2)).reshape(4 * D, H * H1N)
    W1A, W1B, W1C, W1D = (W1f[0:D], W1f[D:2 * D], W1f[2 * D:3 * D],
                          W1f[3 * D:4 * D])
    W12 = np.concatenate([W1A + W1D, W1C], 0).astype(bf16)            # [128,256]
    b1f = np.asarray(b1, np.float32).reshape(H * H1N)
    C = (q @ (W1B - W1D) + b1f).astype(np.float32)                    # [B,256]
    W2bd = np.zeros((H * H1N, H * H2N), np.float32)
    W2a = np.asarray(W2, np.float32)
    for h in range(H):
        W2bd[H1N * h:H1N * (h + 1), H2N * h:H2N * (h + 1)] = W2a[h]
    W2bd = W2bd.astype(bf16)
    W3pad = np.zeros((H * H2N, 32), np.float32)
    W3a = np.asarray(W3, np.float32)
    for h in range(H):
        W3pad[H2N * h:H2N * (h + 1), h] = W3a[h]
    W3pad = W3pad.astype(bf16)

    a1f = np.asarray(a1, np.float32)
    a2f = np.asarray(a2, np.float32)
    pv = np.zeros((128, 6), np.float32)
    pv[:, 0] = np.repeat(a1f[0:2], H1N)
    pv[:, 2] = np.repeat(a1f[2:4], H1N)
    pv[:, 3] = np.repeat(a2f, H2N)
    pv[:, 4] = np.asarray(b2, np.float32).reshape(128)

    kT = np.ascontiguousarray(keys.transpose(0, 2, 1))
    kqT = np.ascontiguousarray((keys * q[:, None, :]).transpose(0, 2, 1))
    X2T = np.concatenate([kT, kqT], 1).astype(bf16)                   # [B,128,S]
    kNW = ((keys.reshape(-1, D) @ np.asarray(Wo, np.float32)
            + np.asarray(bo, np.float32)).reshape(B, S, D)).astype(bf16)

    m4 = (np.asarray(mask, np.float32) - 1.0) * 1e30                  # [B,S]

    sel01 = np.zeros((4, 128), np.float32)
    for j in range(4):
        sel01[j, 32 * j:32 * j + 4] = 1.0
    sel01pad = np.zeros((128, 128), np.float32)
    sel01pad[0:4] = sel01
    m01 = np.zeros((128, 4), np.float32)
    for j in range(4):
        m01[32 * j:32 * j + 4, j] = 1.0
    cb = np.concatenate([
        W12, W2bd[0:128], W2bd[128:256], W3pad, sel01pad, m01],
        axis=1).astype(bf16)                                          # [128,676]

    in_maps = []
    for cix in range(NCORES):
        sl = slice(cix * BL, (cix + 1) * BL)
        x2g = np.ascontiguousarray(
            X2T[sl].reshape(NGRP, 4, 2, 128, S)
            .transpose(0, 3, 1, 2, 4).reshape(NGRP, 128, 1600))
        kng = np.ascontiguousarray(
            kNW[sl].reshape(NGRP, 8, S, D)
            .transpose(0, 2, 1, 3).reshape(NGRP, S, 512))
        mgp = np.ascontiguousarray(
            m4[sl].reshape(NGRP, 4, 400)).astype(bf16)
        Cc = C[sl]                                                    # [256,256]
        ct = np.transpose(Cc.reshape(BL, 2, 128), (1, 2, 0))          # [2,128,256]
        ctb = np.ascontiguousarray(
            np.concatenate([ct[0], ct[1]], axis=1))                   # [128,512]
        in_maps.append({
            "x2g": x2g, "kng": kng, "mgp": mgp,
            "cb": np.ascontiguousarray(cb), "ctb": ctb, "pv": pv,
        })
    return in_maps


_NC_CACHE = {}


def get_nc():
    if "nc" not in _NC_CACHE:
        _NC_CACHE["nc"] = build_nc()
    return _NC_CACHE["nc"]


def kernel(**inputs) -> np.ndarray:
    in_maps = prep_inputs(**inputs)
    nc = get_nc()
    res = run_bass_kernel_spmd(nc, in_maps, core_ids=list(range(NCORES)))
    return np.concatenate([r["out"] for r in res.results], 0)
